# revision 19
# baseline (speedup 1.0000x reference)
"""Masked multi-head attention (B=4, S=2048, H=16, d_k=64) on 8 TRN2 NeuronCores.

Sharding: core c handles batch b = c//2 and head-group hg = c%2 (8 heads each,
processed as 4 pairs: head A on SBUF partitions 0-63, head B on 64-127).

v2 design (empirical HW cost model from micro-benchmarks):
  * scores: bf16 row-tiled matmul pairs (64x128 PE tiles T0/T8 run the two
    heads CONCURRENTLY), N=1024 wide (q-span), PSUM [128,1024] per slot.
  * exp+mask subsystem split across three engines (the bottleneck):
      - ACT path : e_raw = exp(psum/A) on the scalar engine, then
        e = e_raw * mask on DVE or GPSIMD (mask multiply floats freely).
      - DVE path : Schraudolph bits trick fused with the mask:
        e_bits_i16 = round(psum + B), B = 16248*m + 2048*(1-m) (fp16),
        reinterpreted as bf16.  psum holds s*A (A = 128*log2 e; Q is
        pre-scaled by A/8 on the host), so bits = s*log2(e)*128 + 16248
        are exactly the bf16 bits of ~exp(s) (rel err ~3%, bounded).
      - pattern: head-A slots always ACT; head-B slots DVE except 4 kts.
  * attnV: [V | ones] stationary (Z accumulates in PSUM rows 64-127),
    chained over 16 k-tiles, N=1024.  Emission delayed 4 k-tiles so the
    PE interleaves next scores with previous attnV.
  * normalization on the HOST: kernel returns raw numerator rows 0-63 and
    Z row 64 per (head, q); numpy divides.  Saves the Ln/Exp/mul pass.
  * mask (bf16 {0,1}) and bias (fp16) windows resident in SBUF full-q;
    q/k streamed per (pair, span).
"""

import sys

sys.path.insert(0, "/opt/trn_rl_repo")

import numpy as np
import ml_dtypes

import concourse.bass as bass
import concourse.tile as tile
import concourse.mybir as mybir
from concourse import bacc
from concourse import bass_utils

BF16 = mybir.dt.bfloat16
F16 = mybir.dt.float16
F32 = mybir.dt.float32
I16 = mybir.dt.int16

# Model dims
S = 2048
DK = 64
HPC = 8
N_CORES = 8
P = 128
W = 1024            # q-span (PSUM-bank limited)
A_SCALE = 128.0 * np.log2(np.e)   # 184.6644

B_UNMASK = 16248.0  # Schraudolph bias (fp16-exact), sigma=8 centering
B_MASK = 2048.0     # keeps masked bits positive & tiny (~2^-111)

# Per-kt engine pattern (16 k-tiles). Head A slot is always ACT path.
# Head B slot: ACT at these kts, DVE bits-trick otherwise.
ACT_B_KTS = (3, 7, 11, 15)
DVE_KTS = tuple(kt for kt in range(16) if kt not in ACT_B_KTS)
# mask-multiply engine per ACT slot: (kt, head) -> 'pool' | 'dve'
def _mask_eng(kt, head):
    # measured rates want ~60% of the 20 ACT-slot masks on GPSIMD
    if head == 0:
        return "pool" if kt % 4 != 3 else "dve"   # 12 of 16
    return "dve"                                   # all 4 B-ACT slots

DELAY = 6           # attnV emission delay in 512-slots (3 k-tiles)
TRACE = False
LAST_RESULTS = None
DIAG = None         # None | 'pe_only' | 'eng_only'  (timing diagnostics)


def build_program(s=S, hpc=HPC, reps=1):
    kt_n = s // P          # 16
    spans = s // W         # 2
    pairs = hpc // 2       # 4
    hd = hpc * DK          # 512
    n_dve = len(DVE_KTS)

    Exp = mybir.ActivationFunctionType.Exp

    nc = bacc.Bacc("TRN2", debug=False)
    qT = nc.dram_tensor("qT", [hd, s], BF16, kind="ExternalInput").ap()
    kT = nc.dram_tensor("kT", [hd, s], BF16, kind="ExternalInput").ap()
    v = nc.dram_tensor("v", [s, hd], BF16, kind="ExternalInput").ap()
    mT = nc.dram_tensor("mT", [s, s], BF16, kind="ExternalInput").ap()
    bT = nc.dram_tensor("bT", [max(n_dve, 1) * P, s], F16,
                        kind="ExternalInput").ap()
    out_raw = nc.dram_tensor("out_raw", [hpc, 65, s], F32,
                             kind="ExternalOutput").ap()

    with tile.TileContext(nc) as tc:
        with (
            tc.tile_pool(name="resident", bufs=1) as resident,
            tc.tile_pool(name="kwinp", bufs=2) as kwinp,
            tc.tile_pool(name="qwinp", bufs=2) as qwinp,
            tc.tile_pool(name="erawp", bufs=6) as erawp,
            tc.tile_pool(name="ep", bufs=28) as ep,
            tc.tile_pool(name="osbp", bufs=4) as osbp,
            tc.tile_pool(name="psum_s", bufs=4, space="PSUM") as psum_s,
            tc.tile_pool(name="psum_o", bufs=1, space="PSUM") as psum_o,
        ):
            # ---- resident loads (once per NEFF) ----
            # mask window: [128, kt*s] bf16, slice kt at cols kt*s..
            m_sb = resident.tile([P, kt_n * s], BF16)
            for kt in range(kt_n):
                nc.sync.dma_start(m_sb[:, kt * s:(kt + 1) * s],
                                  mT[kt * P:(kt + 1) * P, :])
            # bias window: only DVE kts, [128, n_dve*s] fp16
            if n_dve:
                b_sb = resident.tile([P, n_dve * s], F16)
                for j in range(n_dve):
                    nc.sync.dma_start(b_sb[:, j * s:(j + 1) * s],
                                      bT[j * P:(j + 1) * P, :])
            # v_sb: [128, hpc*kt_n*128]; slot (h, kt) = [V_tile | ones*64]
            v_sb = resident.tile([P, hpc * kt_n * P], BF16)
            v_sb3 = v_sb.rearrange("p (t e) -> p t e", e=P)
            nc.gpsimd.memset(v_sb3[:, :, 64:128], 1.0)
            v_src = v.rearrange("(kt p) c -> p kt c", p=P)
            for h in range(hpc):
                dst = v_sb[:, h * kt_n * P:(h + 1) * kt_n * P]
                dst3 = dst.rearrange("p (kt e) -> p kt e", e=P)
                nc.sync.dma_start(dst3[:, :, 0:64],
                                  v_src[:, :, h * DK:(h + 1) * DK])

            for rep in range(reps):
              for p in range(pairs):
                # K rows for this pair (2 heads stacked 64+64), all kts
                kwin = kwinp.tile([P, s], BF16, tag="kw")
                nc.sync.dma_start(kwin[:], kT[p * P:(p + 1) * P, :])
                for sp in range(spans):
                    qwin = qwinp.tile([P, W], BF16, tag="qw")
                    nc.sync.dma_start(
                        qwin[:], qT[p * P:(p + 1) * P, sp * W:(sp + 1) * W])
                    if DIAG != "eng_only":
                        o_psA = psum_o.tile([P, W], F32, tag="oA")
                        o_psB = psum_o.tile([P, W], F32, tag="oB")
                    hA, hB = 2 * p, 2 * p + 1

                    pending = []   # delayed attnV thunks; slot granularity 512

                    def emit_attnv(kt, hf, eA, eB):
                        def go():
                            cs = slice(hf * 512, (hf + 1) * 512)
                            nc.tensor.matmul(
                                o_psA[:, cs],
                                lhsT=v_sb3[:, hA * kt_n + kt, :],
                                rhs=eA[:],
                                start=(kt == 0), stop=(kt == kt_n - 1))
                            nc.tensor.matmul(
                                o_psB[:, cs],
                                lhsT=v_sb3[:, hB * kt_n + kt, :],
                                rhs=eB[:],
                                start=(kt == 0), stop=(kt == kt_n - 1))
                        return go

                    for kt in range(kt_n):
                      for hf in range(2):
                        # emit delayed attnV FIRST (its e inputs are long
                        # ready) so scores feed the engines ASAP after
                        cs = slice(hf * 512, (hf + 1) * 512)
                        if DIAG != "eng_only" and len(pending) > DELAY:
                            pending.pop(0)()
                        # ---- scores: row-tiled concurrent pair, N=512 ----
                        s_psA = psum_s.tile([P, 512], F32, tag="sps")
                        s_psB = psum_s.tile([P, 512], F32, tag="sps")
                        nc.tensor.matmul(
                            s_psA[:],
                            lhsT=kwin[0:64, kt * P:(kt + 1) * P],
                            rhs=qwin[0:64, cs], start=True, stop=True)
                        nc.tensor.matmul(
                            s_psB[:],
                            lhsT=kwin[64:128, kt * P:(kt + 1) * P],
                            rhs=qwin[64:128, cs], start=True, stop=True)
                        # ---- exp/mask engine ops, 512 wide ----
                        ofs = kt * s + sp * W + hf * 512
                        msl = m_sb[:, ofs:ofs + 512]
                        eA = ep.tile([P, 512], BF16, tag="e")
                        eB = ep.tile([P, 512], BF16, tag="e")
                        if DIAG == "pe_only":
                            if rep == 0 and p == 0 and sp == 0 and kt < 8:
                                nc.gpsimd.memset(eA[:], 0.001)
                                nc.gpsimd.memset(eB[:], 0.001)
                            nc.vector.tensor_copy(eA[0:1, 0:8], s_psA[0:1, 0:8])
                            nc.vector.tensor_copy(eB[0:1, 0:8], s_psB[0:1, 0:8])
                        else:
                            # head A: ACT path
                            erA = erawp.tile([P, 512], BF16, tag="er")
                            nc.scalar.activation(erA[:], s_psA[:], Exp,
                                                 scale=float(1.0 / A_SCALE))
                            engA = nc.gpsimd if _mask_eng(kt, 0) == "pool" \
                                else nc.vector
                            engA.tensor_mul(eA[:], erA[:], msl)
                            # head B: DVE bits path or ACT path
                            if kt in DVE_KTS:
                                j = DVE_KTS.index(kt)
                                bofs = j * s + sp * W + hf * 512
                                bsl = b_sb[:, bofs:bofs + 512]
                                nc.vector.tensor_add(eB[:].bitcast(I16),
                                                     s_psB[:], bsl)
                            else:
                                erB = erawp.tile([P, 512], BF16, tag="er")
                                nc.scalar.activation(erB[:], s_psB[:], Exp,
                                                     scale=float(1.0 / A_SCALE))
                                engB = nc.gpsimd if _mask_eng(kt, 1) == "pool" \
                                    else nc.vector
                                engB.tensor_mul(eB[:], erB[:], msl)
                        if DIAG != "eng_only":
                            pending.append(emit_attnv(kt, hf, eA, eB))
                    for go in pending:
                        go()
                    # ---- out: numerator rows 0-63 + Z row 64, raw ----
                    # (DMA cannot read PSUM: stage via SBUF, alternating the
                    # evacuation engine to spread the cost)
                    o_sbA = osbp.tile([65, W], F32, tag="osb")
                    o_sbB = osbp.tile([65, W], F32, tag="osb")
                    if DIAG == "eng_only":
                        nc.vector.memset(o_sbA[:], 0.0)
                        nc.vector.memset(o_sbB[:], 0.0)
                    elif (p + sp) % 2 == 0:
                        nc.scalar.copy(o_sbA[:], o_psA[0:65, :])
                        nc.vector.tensor_copy(o_sbB[:], o_psB[0:65, :])
                    else:
                        nc.vector.tensor_copy(o_sbA[:], o_psA[0:65, :])
                        nc.scalar.copy(o_sbB[:], o_psB[0:65, :])
                    nc.sync.dma_start(
                        out_raw[hA, :, sp * W:(sp + 1) * W], o_sbA[:])
                    nc.sync.dma_start(
                        out_raw[hB, :, sp * W:(sp + 1) * W], o_sbB[:])
    nc.compile()
    return nc


_PROG = None


def _get_prog():
    global _PROG
    if _PROG is None:
        _PROG = build_program()
    return _PROG


def _prep_in_maps(query, key, value, mask):
    query = np.asarray(query, dtype=np.float32)
    key = np.asarray(key, dtype=np.float32)
    value = np.asarray(value, dtype=np.float32)
    mask = np.asarray(mask)
    B = query.shape[0]
    bf16 = ml_dtypes.bfloat16
    hd = HPC * DK
    n_dve = len(DVE_KTS)

    mTs, bTs = [], []
    for b in range(B):
        mt = np.ascontiguousarray(mask[b, 0].T).astype(np.float32)  # [k, q]
        mTs.append(mt.astype(bf16))
        if DVE_KTS:
            bt = (B_MASK + (B_UNMASK - B_MASK) * mt).astype(np.float16)
            bTs.append(np.concatenate(
                [bt[kt * P:(kt + 1) * P, :] for kt in DVE_KTS], axis=0))
        else:
            bTs.append(np.zeros((P, mt.shape[1]), np.float16))

    q_scale = A_SCALE / 8.0
    in_maps = []
    for c in range(N_CORES):
        b, hg = divmod(c, 2)
        cols = slice(hg * hd, (hg + 1) * hd)
        in_maps.append({
            "qT": np.ascontiguousarray(
                (query[b][:, cols] * q_scale).T).astype(bf16),
            "kT": np.ascontiguousarray(key[b][:, cols].T).astype(bf16),
            "v": value[b][:, cols].astype(bf16),
            "mT": mTs[b],
            "bT": bTs[b],
        })
    return in_maps


def _unshard(results, B, s, D):
    hd = HPC * DK
    out = np.empty((B, s, D), np.float32)
    for c in range(N_CORES):
        b, hg = divmod(c, 2)
        raw = results[c]["out_raw"]          # [8, 65, s]
        num = raw[:, 0:64, :]                # [8, 64, s]
        z = raw[:, 64:65, :]                 # [8, 1, s]
        o = (num / z).transpose(2, 0, 1).reshape(s, hd)   # [s, hd]
        out[b][:, hg * hd:(hg + 1) * hd] = o
    return out


def kernel(query, key, value, mask):
    global LAST_RESULTS
    B, s, D = np.asarray(query).shape
    in_maps = _prep_in_maps(query, key, value, mask)
    nc = _get_prog()
    res = bass_utils.run_bass_kernel_spmd(
        nc, in_maps, core_ids=list(range(N_CORES)), trace=False)
    LAST_RESULTS = res
    return _unshard(res.results, B, s, D)
